# revision 16
# baseline (speedup 1.0000x reference)
"""Trainium2 Bass kernel for nn_MDA_4183298146862 (MDA dense_cnn module).

[2,1024,64,64] -> 32 group slices [64ch,64,64]; 4 per core (2 pairs packed
2-per-128-partitions).  All matmuls bf16 with slice-block-diagonal
stationaries so one matmul serves both packed slices.

DCNv2 via the commuted hat decomposition: for tap k and stencil shift
s=(sy,sx) in the 3x3 core, the per-pixel weight u_{k,s} = m_k*hy(sy)*hx(sx)
multiplies the *input* field X=x2n (shifted by t_k+s); the sums over all 81
(k,s) terms and over channels run inside PSUM accumulation:
    out[o,p] = sum_{k,s,c} Wdcn[o,c,k] * u_{k,s}(p) * X[c, p+t_k+s]
DVE only does one elementwise multiply per term.  u fields are built on
<=81-partition tiles with permutation matmuls (no small-DMA storms),
written to DRAM scratch, and broadcast-replicated across the 64 channel
partitions by DMA at consumption time.  Hat weights relu-clip for |d|>1;
the few high-weight |d|>1 pixel clusters (host-planned, weight>0.02) get
exact extra ring terms accumulated the same way on clipped row ranges.
"""

import numpy as np
import ml_dtypes
from contextlib import ExitStack

import concourse.bass as bass
import concourse.bacc as bacc
import concourse.tile as tile
import concourse.mybir as mybir
from concourse.bass_utils import run_bass_kernel_spmd

F32 = mybir.dt.float32
BF16 = mybir.dt.bfloat16
AF = mybir.ActivationFunctionType
ALU = mybir.AluOpType
AX = mybir.AxisListType

EPS32 = 1.1920929e-07
BN_EPS = 1e-5
GN_EPS = 1e-5
H = W = 64
HW = H * W
NCORES = 8
NSLICES = 4
PAIRS = 2
YCH = 8
NCH = H // YCH
RARE_TH = 0.02

YM = 3
XM = 4
SLAB_H = YM + H + 3      # 70
SLAB_W = XM + W + 4      # 72


# ---------------------------------------------------------------------------
# host-side preprocessing
# ---------------------------------------------------------------------------

def _blkdiag(a):
    """[k,m] -> [2k, 2m] block diagonal."""
    k, m = a.shape
    out = np.zeros((2 * k, 2 * m), np.float32)
    out[:k, :m] = a
    out[k:, m:] = a
    return out


def _host_prep(inputs):
    f = np.float32
    g = lambda n: np.asarray(inputs[n], f)
    w = {}
    bn_s = g("inv_bn_g") / np.sqrt(1.0 + BN_EPS)

    # legacy layouts for the host offset simulation
    w["c3_lhsT"] = np.ascontiguousarray(
        g("c3_w").reshape(64, 64, 9).transpose(1, 2, 0))
    w["c3_b"] = g("c3_b").reshape(64, 1)
    w["gn_g"] = g("gn_g").reshape(64, 1)
    w["gn_b"] = g("gn_b").reshape(64, 1)
    perm = list(range(0, 18, 2)) + list(range(1, 18, 2)) + list(range(18, 27))
    w["off_lhsT"] = np.ascontiguousarray(
        g("off_w")[perm].reshape(27, 64, 9).transpose(1, 2, 0))
    w["off_b"] = g("off_b")[perm].reshape(27, 1)

    # bf16 block-diagonal stationaries
    w["invred_blk"] = _blkdiag(g("inv_reduce_w").T)               # [128,32]
    w["span_blk"] = _blkdiag(g("inv_span_w").T)                   # [32,8]
    rep16 = np.zeros((4, 64), f)
    for i in range(4):
        rep16[i, i * 16:(i + 1) * 16] = 1.0
    w["rep16_blk"] = _blkdiag(rep16)                              # [8,128]
    w["red_blk"] = _blkdiag(g("red_w").T)                         # [128,64]
    w["res_blkf"] = _blkdiag((g("res_w") / 64.0).T)               # [64,128]
    w["fc1_blkf"] = _blkdiag(g("fc1_w").T)                        # [128,32]
    w["fc2_blkf"] = _blkdiag(g("fc2_w").T)                        # [32,128]
    c3T = w["c3_lhsT"]                                            # [64,9,64]
    w["c3_blk"] = np.stack([_blkdiag(c3T[:, t, :]) for t in range(9)],
                           1).reshape(128, 9 * 128)
    offT = w["off_lhsT"]                                          # [64,9,27]
    colmap = np.zeros(54, np.int64)        # old blockdiag col -> new row
    for sl in range(2):
        for t in range(9):
            colmap[sl * 27 + 0 + t] = 18 + sl * 18 + t      # dy
            colmap[sl * 27 + 9 + t] = 27 + sl * 18 + t      # dx
            colmap[sl * 27 + 18 + t] = sl * 9 + t           # mask
    ob = np.stack([_blkdiag(offT[:, t, :]) for t in range(9)], 1)  # [128,9,54]
    ob2 = np.zeros_like(ob)
    ob2[:, :, colmap] = ob
    w["off_blk"] = ob2.reshape(128, 9 * 54)
    dcnT = np.ascontiguousarray(
        g("dcn_w").reshape(64, 64, 9).transpose(1, 2, 0))         # [64,9,64]
    w["dcn_blk"] = np.stack([_blkdiag(dcnT[:, t, :]) for t in range(9)],
                            1).reshape(128, 9 * 128)

    # permutation stationaries for hat-field construction
    def prep3(kind):
        p = np.zeros((54, 54), f)
        for sl in range(2):
            for grp in range(3):
                for t in range(9):
                    if kind == "y":
                        src = 18 + sl * 18 + t
                    elif kind == "x":
                        src = 27 + sl * 18 + t
                    else:
                        src = sl * 9 + t
                    p[src, sl * 27 + grp * 9 + t] = 1.0
        return p
    w["P_repy"] = prep3("y")
    w["P_repx"] = prep3("x")
    w["P_repm"] = prep3("m")
    for sl in range(2):
        pa = np.zeros((54, 81), f)
        px = np.zeros((54, 81), f)
        for sy in range(3):
            for sx in range(3):
                for t in range(9):
                    r = (sy * 3 + sx) * 9 + t
                    pa[sl * 27 + sy * 9 + t, r] = 1.0
                    px[sl * 27 + sx * 9 + t, r] = 1.0
        w[f"P_ayS{sl}"] = pa
        w[f"P_axS{sl}"] = px

    # f32 bias/const columns
    w["inv_scale_pk"] = np.tile(bn_s, 2).reshape(32, 1)
    w["inv_bias_pk"] = np.tile(bn_s * g("inv_reduce_b") + g("inv_bn_b"),
                               2).reshape(32, 1)
    w["span_b_pk"] = np.tile(g("inv_span_b"), 2).reshape(8, 1)
    red_b = g("red_b") + EPS32 * g("red_w").sum(1)
    w["red_b_pk"] = np.tile(red_b, 2).reshape(64, 1)
    w["res_b_pk"] = np.tile(g("res_b"), 2).reshape(128, 1)
    w["c3_b_pk"] = np.tile(g("c3_b"), 2).reshape(128, 1)
    w["gn_g_pk"] = np.tile(g("gn_g"), 2).reshape(128, 1)
    w["gn_b_pk"] = np.tile(g("gn_b"), 2).reshape(128, 1)
    ob_b = np.zeros(54, f)
    ob_b[colmap] = np.tile(g("off_b")[perm], 2)
    w["off_b_pk"] = ob_b.reshape(54, 1)
    w["dcn_b_pk"] = np.tile(g("dcn_b"), 2).reshape(128, 1)
    sy_b = np.concatenate([np.repeat([1.0, 0.0, -1.0], 9)] * 2)
    w["sy_bias54"] = sy_b.reshape(54, 1)
    w["ones54"] = np.ones((54, 1), f)
    rb = np.zeros(54, f)
    for sl in range(2):
        rb[sl * 27 + 0:sl * 27 + 9] = -2.0    # sign +2 rows: |d - 2|
        rb[sl * 27 + 9:sl * 27 + 18] = 2.0    # sign -2 rows: |d + 2|
    w["rare_b54"] = rb.reshape(54, 1)
    return w


def _host_offsets(x_slices, wd):
    """Offset fields [S, 27, H, W] on host (fp32 sim of device pipeline)."""
    S = x_slices.shape[0]
    xs = x_slices.reshape(S, 64, H, W).astype(np.float32)

    def conv3x3(inp, lhsT, nout):
        pad = np.zeros((S, 64, H + 2, W + 2), np.float32)
        pad[:, :, 1:-1, 1:-1] = inp
        out = np.zeros((S, nout, H, W), np.float32)
        for t in range(9):
            ty, tx = t // 3, t % 3
            win = pad[:, :, ty:ty + H, tx:tx + W]
            out += np.einsum("co,schw->sohw", lhsT[:, t, :], win,
                             optimize=True)
        return out

    xc3 = conv3x3(xs, wd["c3_lhsT"], 64) + wd["c3_b"].reshape(1, 64, 1, 1)
    mu = xc3.mean(axis=(2, 3), keepdims=True)
    var = xc3.var(axis=(2, 3), keepdims=True)
    x2n = ((xc3 - mu) / np.sqrt(var + GN_EPS)
           * wd["gn_g"].reshape(1, 64, 1, 1) + wd["gn_b"].reshape(1, 64, 1, 1))
    return conv3x3(x2n, wd["off_lhsT"], 27) + wd["off_b"].reshape(1, 27, 1, 1)


def _plan_rare(off_fields, th=RARE_TH):
    """Per pair: list of (sl, k, sy, sx, ya, yb, akind, bkind) for ring terms
    with max hat-product weight > th.  akind: ('core', sy) or ('rare', sign);
    bkind likewise for x."""
    S = off_fields.shape[0]
    sig = 1.0 / (1.0 + np.exp(-off_fields[:, 18:27]))
    plans = [[] for _ in range(S // 2)]
    for s in range(S):
        pair = s // 2
        sl = s % 2
        for k in range(9):
            dy = off_fields[s, k]
            dx = off_fields[s, 9 + k]
            m = sig[s, k]
            for sy in (-2, -1, 0, 1, 2):
                hy = np.maximum(0.0, 1.0 - np.abs(dy - sy))
                for sx in (-2, -1, 0, 1, 2):
                    if abs(sy) <= 1 and abs(sx) <= 1:
                        continue
                    hx = np.maximum(0.0, 1.0 - np.abs(dx - sx))
                    wf = hy * hx * m
                    if wf.max() <= th:
                        continue
                    rows = np.nonzero((wf > th).any(axis=1))[0]
                    ya, yb = int(rows[0]), int(rows[-1] + 1)
                    ak = ('core', sy) if abs(sy) <= 1 else ('rare', sy // 2)
                    bk = ('core', sx) if abs(sx) <= 1 else ('rare', sx // 2)
                    plans[pair].append((sl, k, sy, sx, ya, yb, ak, bk))
    return plans


# ---------------------------------------------------------------------------
# weight blobs
# ---------------------------------------------------------------------------

_F32_SPEC = [
    ("inv_scale_pk", 1, 32), ("inv_bias_pk", 1, 32), ("span_b_pk", 1, 8),
    ("red_b_pk", 1, 64), ("res_b_pk", 1, 128), ("c3_b_pk", 1, 128),
    ("gn_g_pk", 1, 128), ("gn_b_pk", 1, 128), ("off_b_pk", 1, 54),
    ("dcn_b_pk", 1, 128), ("sy_bias54", 1, 54), ("ones54", 1, 54),
    ("rare_b54", 1, 54), ("res_blkf", 128, 64), ("fc1_blkf", 32, 128),
    ("fc2_blkf", 128, 32),
]

_BF16_SPEC = [
    ("invred_blk", 32, 128), ("span_blk", 8, 32), ("rep16_blk", 128, 8),
    ("red_blk", 64, 128), ("c3_blk", 9 * 128, 128), ("off_blk", 9 * 54, 128),
    ("dcn_blk", 9 * 128, 128), ("P_repy", 54, 54), ("P_repx", 54, 54),
    ("P_repm", 54, 54), ("P_ayS0", 81, 54), ("P_ayS1", 81, 54),
    ("P_axS0", 81, 54), ("P_axS1", 81, 54),
]


def _rare_pmats(plans):
    """Selector stationaries for rare-u row stacks, per pair.
    A-side sources ay54 (core) / rf_y (rare); B-side hx54 / rf_x.
    rf tiles: rows sl*27 + (0 if sign>0 else 9) + t."""
    mats = {}
    for p, entries in enumerate(plans):
        E = max(1, len(entries))
        pca = np.zeros((54, E), np.float32)
        pra = np.zeros((54, E), np.float32)
        pcb = np.zeros((54, E), np.float32)
        prb = np.zeros((54, E), np.float32)
        for e, (sl, k, sy, sx, ya, yb, ak, bk) in enumerate(entries):
            if ak[0] == 'core':
                pca[sl * 27 + (ak[1] + 1) * 9 + k, e] = 1.0
            else:
                pra[sl * 27 + (0 if ak[1] > 0 else 9) + k, e] = 1.0
            if bk[0] == 'core':
                pcb[sl * 27 + (bk[1] + 1) * 9 + k, e] = 1.0
            else:
                prb[sl * 27 + (0 if bk[1] > 0 else 9) + k, e] = 1.0
        mats[f"P_rcA{p}"] = pca
        mats[f"P_rrA{p}"] = pra
        mats[f"P_rcB{p}"] = pcb
        mats[f"P_rrB{p}"] = prb
    return mats


def _build_specs(plans):
    f32_spec = list(_F32_SPEC)
    bf16_spec = list(_BF16_SPEC)
    for p in range(PAIRS):
        E = max(1, len(plans[p]))
        for nm in (f"P_rcA{p}", f"P_rrA{p}", f"P_rcB{p}", f"P_rrB{p}"):
            bf16_spec.append((nm, E, 54))
    return f32_spec, bf16_spec


def _cols(spec):
    cols = {}
    o = 0
    for name, ncols, kdim in spec:
        cols[name] = (o, ncols, kdim)
        o += ncols
    return cols


def _build_blobs(wd, plans):
    f32_spec, bf16_spec = _build_specs(plans)
    rmats = _rare_pmats(plans)
    c1 = _cols(f32_spec)
    n1 = sum(n for _, n, _ in f32_spec)
    b1 = np.zeros((128, n1), np.float32)
    for name, (o, ncols, kdim) in c1.items():
        b1[0:kdim, o:o + ncols] = wd[name].reshape(kdim, ncols)
    c2 = _cols(bf16_spec)
    n2 = sum(n for _, n, _ in bf16_spec)
    b2 = np.zeros((128, n2), np.float32)
    for name, (o, ncols, kdim) in c2.items():
        arr = rmats[name] if name in rmats else wd[name]
        b2[0:kdim, o:o + ncols] = arr.reshape(kdim, ncols)
    return b1, b2.astype(ml_dtypes.bfloat16), c1, c2, n1, n2


# ---------------------------------------------------------------------------
# bass program
# ---------------------------------------------------------------------------

def build_nc(wd, plans, repeat=1, debug=False):
    b1, b2, c1, c2, n1, n2 = _build_blobs(wd, plans)
    nc = bacc.Bacc("TRN2", target_bir_lowering=False, debug=debug)
    xin = nc.dram_tensor("xin", [NSLICES, 64, HW], F32,
                         kind="ExternalInput").ap()
    xin16 = nc.dram_tensor("xin16", [NSLICES, 64, HW], BF16,
                           kind="ExternalInput").ap()
    yout = nc.dram_tensor("yout", [NSLICES, 64, HW], F32,
                          kind="ExternalOutput").ap()
    wb1 = nc.dram_tensor("wblob", [128, n1], F32, kind="ExternalInput").ap()
    wb2 = nc.dram_tensor("wblob16", [128, n2], BF16,
                         kind="ExternalInput").ap()
    uscr = [nc.dram_tensor(f"uscr{p}", [2, 81, HW], BF16).ap()
            for p in range(PAIRS)]
    rscr = [nc.dram_tensor(f"rscr{p}", [max(1, len(plans[p])), HW], BF16).ap()
            for p in range(PAIRS)]

    st = {"bc": 0}   # broadcast engine rotation

    with tile.TileContext(nc) as tc:
        with ExitStack() as ctx:
            tp = lambda **kw: ctx.enter_context(tc.tile_pool(**kw))
            consts = tp(name="consts", bufs=1)
            pers = tp(name="pers", bufs=1)
            big = tp(name="big", bufs=1)
            t128 = tp(name="t128", bufs=2)      # xc3 / xr2d rotation
            chk = tp(name="chk", bufs=2)        # conv chunk stages
            hat = tp(name="hat", bufs=2)        # hat-build 512-col tiles
            smalls = tp(name="smalls", bufs=2)
            ph2p = tp(name="ph2p", bufs=3)
            utp = tp(name="utp", bufs=6)
            offp = tp(name="offp", bufs=2)
            ph3p = tp(name="ph3p", bufs=1)
            psum1 = tp(name="psum1", bufs=4, space="PSUM")
            psamp = tp(name="psamp", bufs=1, space="PSUM")

            blob1 = consts.tile([128, n1], F32, tag="b1", name="b1")
            blob2 = consts.tile([128, n2], BF16, tag="b2", name="b2")
            nc.sync.dma_start(blob1[:], wb1[:])
            nc.sync.dma_start(blob2[:], wb2[:])
            ccols = {}
            for v in (GN_EPS,):
                t = consts.tile([128, 1], F32, tag=f"cc{v}", name=f"cc{v}")
                nc.gpsimd.memset(t[:], float(v))
                ccols[float(v)] = t

            wt = {"b1": blob1, "b2": blob2, "c1": c1, "c2": c2,
                  "ccols": ccols}

            tc.strict_bb_all_engine_barrier()

            pair_t = []
            for p in range(PAIRS):
                pair_t.append({
                    "x2ne": pers.tile([128, SLAB_H, SLAB_W], BF16,
                                      tag=f"x2ne{p}", name=f"x2ne{p}"),
                    "x2no": pers.tile([128, SLAB_H, SLAB_W - 1], BF16,
                                      tag=f"x2no{p}", name=f"x2no{p}"),
                    "out0": pers.tile([128, HW], BF16, tag=f"out0{p}",
                                      name=f"out0{p}"),
                    "ca": pers.tile([128, 1], F32, tag=f"ca{p}",
                                    name=f"ca{p}"),
                })

            gxp = tp(name="gxp", bufs=2)
            for rep in range(repeat):
                args = (tc, nc, xin16, wt, plans, uscr, rscr,
                        big, t128, chk, hat, smalls, psum1, st, gxp, offp,
                        utp)
                gx0 = _ph1_load(nc, 0, pair_t[0], xin16, gxp)
                gx1 = _ph1_load(nc, 1, pair_t[1], xin16, gxp)
                u0 = _ph1_dcn_units(0, pair_t[0], gx0, *args)
                u1 = _ph1_dcn_units(1, pair_t[1], gx1, *args)
                for f in u0:
                    f()
                r0 = _ph1_rest_units(0, pair_t[0], gx0, *args)
                r1 = _ph1_rest_units(1, pair_t[1], gx1, *args)
                xr2d0 = t128.tile([128, HW], BF16, tag="t128", name="xr2d0")
                _ph2(tc, nc, 0, pair_t[0], xr2d0, wt, plans, uscr, rscr,
                     ph2p, psamp, st, interleave=u1 + r0, utp=utp)
                _ph3(tc, nc, 0, pair_t[0], xr2d0, xin, yout, ph3p)
                xr2d1 = t128.tile([128, HW], BF16, tag="t128", name="xr2d1")
                _ph2(tc, nc, 1, pair_t[1], xr2d1, wt, plans, uscr, rscr,
                     ph2p, psamp, st, interleave=r1, utp=utp)
                _ph3(tc, nc, 1, pair_t[1], xr2d1, xin, yout, ph3p)
    nc.compile()
    return nc


def _w1(wt, name, nparts=None):
    o, ncols, kdim = wt["c1"][name]
    return wt["b1"][0:(nparts or kdim), o:o + ncols]


def _w2(wt, name):
    o, ncols, kdim = wt["c2"][name]
    ap = wt["b2"][0:kdim, o:o + ncols]
    if ncols > 160:
        ap = ap.rearrange("k (t m) -> k t m", t=9)
    return ap


def _bcast_dma(nc, st, out_ap, in_ap):
    eng = (nc.sync, nc.scalar, nc.sync, nc.gpsimd)[st["bc"] % 4]
    st["bc"] += 1
    eng.dma_start(out_ap, in_ap)


def _zero_margins(nc, slab, wdt):
    nc.gpsimd.memset(slab[:, 0:YM, :], 0.0)
    nc.gpsimd.memset(slab[:, YM + H:SLAB_H, :], 0.0)
    nc.gpsimd.memset(slab[:, YM:YM + H, 0:XM], 0.0)
    nc.gpsimd.memset(slab[:, YM:YM + H, XM + W:wdt], 0.0)


def _ph1_load(nc, p, pt, xin16, gxp):
    s0 = 2 * p
    gx = gxp.tile([128, SLAB_H, SLAB_W], BF16, tag="gx", name=f"gx{p}")
    _zero_margins(nc, gx, SLAB_W)
    _zero_margins(nc, pt["x2ne"], SLAB_W)
    for sl in range(2):
        nc.sync.dma_start(gx[64 * sl:64 * sl + 64, YM:YM + H, XM:XM + W],
                          xin16[s0 + sl].rearrange("c (h w) -> c h w", w=W))
    return gx


def _gxw(gx, ch, dy=0, dx=0):
    return gx[:, YM + ch * YCH + dy:YM + ch * YCH + dy + YCH,
              XM + dx:XM + dx + W]


def _ph1_dcn_units(p, pt, gx, tc, nc, xin16, wt, plans, uscr, rscr,
                   big, t128, chk, hat, smalls, psum1, st, gxp, offp, utp):
    """Closures (in order): 8x c3-chunk, GN, 8x off-chunk, 8x hat-chunk."""
    entries = plans[p]
    x2ne = pt["x2ne"]
    xc3 = t128.tile([128, HW], BF16, tag="t128", name=f"xc3{p}")
    offpk = offp.tile([54, HW], BF16, tag="off", name=f"off{p}")
    sumc = smalls.tile([128, NCH], F32, tag="sumc", name=f"sumc{p}")
    sqc = smalls.tile([128, NCH], F32, tag="sqc", name=f"sqc{p}")
    units = []

    def c3_chunk(ch):
        def f():
            cols = slice(ch * 512, (ch + 1) * 512)
            pc = psum1.tile([128, 512], F32, tag="pc", name="pcE")
            for t in range(9):
                ty, tx = t // 3, t % 3
                nc.tensor.matmul(pc[:], _w2(wt, "c3_blk")[:, t, :],
                                 _gxw(gx, ch, ty - 1, tx - 1),
                                 start=(t == 0), stop=(t == 8))
            nc.scalar.activation(xc3[:, cols], pc[:], AF.Identity,
                                 bias=_w1(wt, "c3_b_pk"),
                                 accum_out=sumc[:, ch:ch + 1])
            scr = chk.tile([128, 512], BF16, tag="scr", name="scr")
            nc.scalar.activation(scr[:], xc3[:, cols], AF.Square,
                                 accum_out=sqc[:, ch:ch + 1])
        return f

    def gn_apply():
        mu = smalls.tile([128, 1], F32, tag="mu", name="mu")
        nc.vector.tensor_reduce(mu[:], sumc[:], AX.X, ALU.add)
        nc.scalar.activation(mu[:], mu[:], AF.Identity, scale=1.0 / HW)
        vr = smalls.tile([128, 1], F32, tag="vr", name="vr")
        nc.vector.tensor_reduce(vr[:], sqc[:], AX.X, ALU.add)
        nc.scalar.activation(vr[:], vr[:], AF.Identity, scale=1.0 / HW)
        ms = smalls.tile([128, 1], F32, tag="ms", name="ms")
        nc.gpsimd.tensor_tensor(ms[:], mu[:], mu[:], ALU.mult)
        nc.gpsimd.tensor_sub(vr[:], vr[:], ms[:])
        nc.scalar.activation(vr[:], vr[:], AF.Sqrt,
                             bias=wt["ccols"][GN_EPS][0:128, :])
        istd = smalls.tile([128, 1], F32, tag="istd", name="istd")
        nc.vector.reciprocal(istd[:], vr[:])
        sc = smalls.tile([128, 1], F32, tag="sc", name="sc")
        nc.gpsimd.tensor_tensor(sc[:], istd[:], _w1(wt, "gn_g_pk"), ALU.mult)
        bi = smalls.tile([128, 1], F32, tag="bi", name="bi")
        nc.gpsimd.tensor_tensor(bi[:], mu[:], sc[:], ALU.mult)
        nc.gpsimd.tensor_sub(bi[:], _w1(wt, "gn_b_pk"), bi[:])
        nc.scalar.activation(x2ne[:, YM:YM + H, XM:XM + W],
                             xc3[:].rearrange("c (h w) -> c h w", w=W),
                             AF.Identity, bias=bi[:], scale=sc[:])

    def x2no_copy():
        nc.scalar.activation(pt["x2no"][:], x2ne[:, :, 1:SLAB_W],
                             AF.Identity)

    def x2w(ch, dy=0, dx=0):
        return x2ne[:, YM + ch * YCH + dy:YM + ch * YCH + dy + YCH,
                    XM + dx:XM + dx + W]

    def off_chunk(ch):
        def f():
            pc = psum1.tile([128, 512], F32, tag="pc", name="pcI")
            for t in range(9):
                ty, tx = t // 3, t % 3
                nc.tensor.matmul(pc[0:54, :], _w2(wt, "off_blk")[:, t, :],
                                 x2w(ch, ty - 1, tx - 1),
                                 start=(t == 0), stop=(t == 8))
            cols = slice(ch * 512, (ch + 1) * 512)
            nc.scalar.activation(offpk[:, cols], pc[0:54, :], AF.Identity,
                                 bias=_w1(wt, "off_b_pk"))
            nc.scalar.activation(offpk[0:18, cols], offpk[0:18, cols],
                                 AF.Sigmoid)
        return f

    have_rare = len(entries) > 0

    def hat_chunk(ch):
        def f():
            cols = slice(ch * 512, (ch + 1) * 512)
            opk = offpk[:, cols]
            pd = psum1.tile([128, 512], F32, tag="pc", name="pcJ")
            nc.tensor.matmul(pd[0:54, :], _w2(wt, "P_repy"), opk,
                             start=True, stop=True)
            t54 = hat.tile([54, 512], BF16, tag="h54", name="t54")
            nc.scalar.activation(t54[:], pd[0:54, :], AF.Abs,
                                 bias=_w1(wt, "sy_bias54"))
            hy54 = hat.tile([54, 512], BF16, tag="h54", name="hy54")
            nc.scalar.activation(hy54[:], t54[:], AF.Relu,
                                 bias=_w1(wt, "ones54"), scale=-1.0)
            pm = psum1.tile([128, 512], F32, tag="pc", name="pcK")
            nc.tensor.matmul(pm[0:54, :], _w2(wt, "P_repm"), opk,
                             start=True, stop=True)
            ay54 = hat.tile([54, 512], BF16, tag="ay", name="ay54")
            nc.vector.tensor_tensor(ay54[:], hy54[:], pm[0:54, :], ALU.mult)
            pdx = psum1.tile([128, 512], F32, tag="pc", name="pcL")
            nc.tensor.matmul(pdx[0:54, :], _w2(wt, "P_repx"), opk,
                             start=True, stop=True)
            t54x = hat.tile([54, 512], BF16, tag="h54", name="t54x")
            nc.scalar.activation(t54x[:], pdx[0:54, :], AF.Abs,
                                 bias=_w1(wt, "sy_bias54"))
            hx54 = hat.tile([54, 512], BF16, tag="hx", name="hx54")
            nc.scalar.activation(hx54[:], t54x[:], AF.Relu,
                                 bias=_w1(wt, "ones54"), scale=-1.0)
            if have_rare:
                rfy = hat.tile([54, 512], BF16, tag="rfy", name="rfy")
                nc.scalar.activation(rfy[0:45, :], pd[0:45, :], AF.Abs,
                                     bias=_w1(wt, "rare_b54", 45))
                nc.scalar.activation(rfy[0:45, :], rfy[0:45, :], AF.Relu,
                                     bias=_w1(wt, "ones54", 45), scale=-1.0)
                nc.vector.tensor_tensor(rfy[0:45, :], rfy[0:45, :],
                                        pm[0:45, :], ALU.mult)
                rfx = hat.tile([54, 512], BF16, tag="rfx", name="rfx")
                nc.scalar.activation(rfx[0:45, :], pdx[0:45, :], AF.Abs,
                                     bias=_w1(wt, "rare_b54", 45))
                nc.scalar.activation(rfx[0:45, :], rfx[0:45, :], AF.Relu,
                                     bias=_w1(wt, "ones54", 45), scale=-1.0)
                E = len(entries)
                pA = psum1.tile([128, 512], F32, tag="pc", name="pcM")
                nc.tensor.matmul(pA[0:E, :], _w2(wt, f"P_rcA{p}"), ay54[:],
                                 start=True, stop=False)
                nc.tensor.matmul(pA[0:E, :], _w2(wt, f"P_rrA{p}"), rfy[:],
                                 start=False, stop=True)
                pB = psum1.tile([128, 512], F32, tag="pc", name="pcN")
                nc.tensor.matmul(pB[0:E, :], _w2(wt, f"P_rcB{p}"), hx54[:],
                                 start=True, stop=False)
                nc.tensor.matmul(pB[0:E, :], _w2(wt, f"P_rrB{p}"), rfx[:],
                                 start=False, stop=True)
                bE = hat.tile([54, 512], BF16, tag="bE", name="bE")
                nc.scalar.activation(bE[0:E, :], pB[0:E, :], AF.Identity)
                ur = hat.tile([54, 512], BF16, tag="ur", name="ur")
                nc.vector.tensor_tensor(ur[0:E, :], pA[0:E, :], bE[0:E, :],
                                        ALU.mult)
                nc.gpsimd.dma_start(rscr[p][:, cols], ur[0:E, :])
            for sl in range(2):
                pa = psum1.tile([128, 512], F32, tag="pc", name="pcO")
                nc.tensor.matmul(pa[0:81, :], _w2(wt, f"P_ayS{sl}"),
                                 ay54[:], start=True, stop=True)
                px = psum1.tile([128, 512], F32, tag="pc", name="pcP")
                nc.tensor.matmul(px[0:81, :], _w2(wt, f"P_axS{sl}"),
                                 hx54[:], start=True, stop=True)
                axc = hat.tile([81, 512], BF16, tag="axc", name="axc")
                nc.scalar.activation(axc[:], px[0:81, :], AF.Identity)
                u81 = hat.tile([81, 512], BF16, tag="u81", name="u81")
                nc.vector.tensor_tensor(u81[:], pa[0:81, :], axc[:],
                                        ALU.mult)
                nc.gpsimd.dma_start(uscr[p][sl, :, cols], u81[:])
        return f

    for ch in range(NCH):
        units.append(c3_chunk(ch))
    units.append(gn_apply)
    for ch in range(NCH):
        units.append(off_chunk(ch))
        units.append(hat_chunk(ch))
        if ch == 0:
            units.append(x2no_copy)
    return units


def _ph1_rest_units(p, pt, gx, tc, nc, xin16, wt, plans, uscr, rscr,
                    big, t128, chk, hat, smalls, psum1, st, gxp, offp, utp):
    """Involution + coordinate attention + channel attention closures."""
    xr = big.tile([64, HW], BF16, tag="xr", name=f"xr{p}")
    units = []

    def inv_chunk(ch):
        def f():
            cols = slice(ch * 512, (ch + 1) * 512)
            pc = psum1.tile([128, 512], F32, tag="pc", name="pcA")
            nc.tensor.matmul(pc[0:32, :], _w2(wt, "invred_blk"),
                             _gxw(gx, ch), start=True, stop=True)
            r_ch = chk.tile([32, 512], BF16, tag="r", name="r")
            nc.scalar.activation(r_ch[:], pc[0:32, :], AF.Relu,
                                 bias=_w1(wt, "inv_bias_pk"),
                                 scale=_w1(wt, "inv_scale_pk"))
            pc = psum1.tile([128, 512], F32, tag="pc", name="pcB")
            nc.tensor.matmul(pc[0:8, :], _w2(wt, "span_blk"), r_ch[:],
                             start=True, stop=True)
            wm_ch = chk.tile([8, 512], BF16, tag="wm", name="wm")
            nc.scalar.activation(wm_ch[:], pc[0:8, :], AF.Identity,
                                 bias=_w1(wt, "span_b_pk"))
            pc = psum1.tile([128, 512], F32, tag="pc", name="pcC")
            nc.tensor.matmul(pc[:], _w2(wt, "rep16_blk"), wm_ch[:],
                             start=True, stop=True)
            xr1_ch = chk.tile([128, 512], BF16, tag="xr1", name="xr1")
            nc.vector.tensor_tensor(
                xr1_ch[:].rearrange("c (a b) -> c a b", b=W),
                pc[:].rearrange("c (a b) -> c a b", b=W), _gxw(gx, ch),
                ALU.mult)
            pc = psum1.tile([128, 512], F32, tag="pc", name="pcD")
            nc.tensor.matmul(pc[0:64, :], _w2(wt, "red_blk"), xr1_ch[:],
                             start=True, stop=True)
            nc.scalar.activation(xr[:, cols], pc[0:64, :], AF.Identity,
                                 bias=_w1(wt, "red_b_pk"))
        return f

    def coord():
        cat = smalls.tile([64, 128], F32, tag="cat", name="cat")
        xr3 = xr[:].rearrange("c (h w) -> c h w", w=W)
        nc.vector.tensor_reduce(cat[:, 0:64], xr3, AX.X, ALU.add)
        nc.vector.tensor_reduce(cat[:, 64:128], xr3.transpose([0, 2, 1]),
                                AX.X, ALU.add)
        pc = psum1.tile([128, 512], F32, tag="pc", name="pcF")
        nc.tensor.matmul(pc[:, 0:128], _w1(wt, "res_blkf"), cat[:],
                         start=True, stop=True)
        hw_pk = smalls.tile([128, 128], BF16, tag="hw", name="hw")
        nc.scalar.activation(hw_pk[:], pc[:, 0:128], AF.Sigmoid,
                             bias=_w1(wt, "res_b_pk"))
        sh_pk = smalls.tile([128, 64], BF16, tag="sh", name="sh")
        nc.scalar.activation(sh_pk[:], hw_pk[:, 0:64], AF.Sigmoid)
        nc.vector.tensor_tensor(
            pt["out0"][:].rearrange("c (h w) -> c h w", w=W),
            gx[:, YM:YM + H, XM:XM + W],
            sh_pk[:, :, None].broadcast_to([128, 64, 64]), ALU.mult)

    def chattn():
        am = smalls.tile([128, 2], F32, tag="am", name="am")
        o0f = pt["out0"][:]
        nc.vector.tensor_reduce(am[:, 0:1], o0f, AX.X, ALU.add)
        nc.vector.tensor_reduce(am[:, 1:2], o0f, AX.X, ALU.max)
        nc.scalar.activation(am[:, 0:1], am[:, 0:1], AF.Identity,
                             scale=1.0 / HW)
        pc = psum1.tile([128, 512], F32, tag="pc", name="pcG")
        nc.tensor.matmul(pc[0:32, 0:2], _w1(wt, "fc1_blkf"), am[:],
                         start=True, stop=True)
        fcr = smalls.tile([32, 2], F32, tag="fcr", name="fcr")
        nc.scalar.activation(fcr[:], pc[0:32, 0:2], AF.Relu)
        pc = psum1.tile([128, 512], F32, tag="pc", name="pcH")
        nc.tensor.matmul(pc[:, 0:2], _w1(wt, "fc2_blkf"), fcr[:],
                         start=True, stop=True)
        cs = smalls.tile([128, 1], F32, tag="cs", name="cs")
        nc.vector.tensor_reduce(cs[:], pc[:, 0:2], AX.X, ALU.add)
        nc.scalar.activation(pt["ca"][:], cs[:], AF.Sigmoid)

    for ch in range(NCH):
        units.append(inv_chunk(ch))
    units.append(coord)
    units.append(chattn)
    return units


def _xwin(pt, row0, col, nrows):
    """Window [128, nrows, 64] at slab row row0, col (absolute incl. margin),
    choosing the even/odd slab for 4B-aligned bf16 starts."""
    if col % 2 == 0:
        return pt["x2ne"][:, row0:row0 + nrows, col:col + W]
    return pt["x2no"][:, row0:row0 + nrows, col - 1:col - 1 + W]


def _ph2(tc, nc, p, pt, xr2d, wt, plans, uscr, rscr, ph2p, psamp, st,
         interleave=(), utp=None):
    entries = plans[p]
    dcn = _w2(wt, "dcn_blk")
    pending = list(interleave)
    slots = [18]

    def pop_units():
        if pending and slots[0] > 0:
            n = max(1, (len(pending) + slots[0] - 1) // slots[0])
            for _ in range(n):
                if pending:
                    pending.pop(0)()
        slots[0] -= 1

    for half in range(2):
        banks = [psamp.tile([128, 1024], F32, tag=f"bk{c}", name=f"bk{c}")
                 for c in range(2)]
        # clip rare entries to this half
        clips = []
        for e, (sl, k, sy, sx, ya, yb, ak, bk) in enumerate(entries):
            a = max(ya, 32 * half)
            b = min(yb, 32 * half + 32)
            if a < b:
                clips.append((e, sl, k, sy, sx, a, b))
        nterms = 81 * 4 + sum(1 for _ in clips)  # per-bank counting via ti
        ti = 0
        last_core = (8, 8)
        for k in range(9):
            ky, kx = k // 3 - 1, k % 3 - 1
            if k == 8:
                for (e, sl, k_e, sy, sx, a, b) in clips:
                    ke_y, ke_x = k_e // 3 - 1, k_e % 3 - 1
                    ny = b - a
                    ubc = ph2p.tile([128, 32, W], BF16, tag="ubc",
                                    name="ubc")
                    _bcast_dma(nc, st,
                               ubc[64 * sl:64 * sl + 64, 0:ny, :],
                               rscr[p][e:e + 1, a * W:b * W]
                               .rearrange("o (h w) -> o h w", w=W)
                               .partition_broadcast(64))
                    cpr = ph2p.tile([128, 32, W], BF16, tag="cpr",
                                    name="cpr")
                    xw = _xwin(pt, YM + a + ke_y + sy, XM + ke_x + sx, ny)
                    nc.vector.tensor_tensor(
                        cpr[64 * sl:64 * sl + 64, 0:ny, :],
                        ubc[64 * sl:64 * sl + 64, 0:ny, :],
                        xw[64 * sl:64 * sl + 64, :, :], ALU.mult)
                    r0 = a - 32 * half
                    r1 = b - 32 * half
                    q0, q1 = r0 // 8, (r1 - 1) // 8
                    for q in range(q0, q1 + 1):
                        ra = max(r0, q * 8)
                        rb = min(r1, q * 8 + 8)
                        c = q // 2
                        nc.tensor.matmul(
                            banks[c][:, (ra - c * 16) * W:(rb - c * 16) * W],
                            dcn[:, k_e, :][64 * sl:64 * sl + 64, :],
                            cpr[64 * sl:64 * sl + 64,
                                ra - r0:rb - r0, :],
                            start=False, stop=False)
            for s in range(9):
                sy, sx = s // 3 - 1, s % 3 - 1
                r = s * 9 + k
                ut = utp.tile([128, 32, W], BF16, tag="ut", name="ut")
                _bcast_dma(nc, st, ut[:],
                           uscr[p][:, r:r + 1,
                                   half * 2048:half * 2048 + 2048]
                           .rearrange("s o (h w) -> s o h w", w=W)
                           .broadcast_to([2, 64, 32, W]))
                prod = ph2p.tile([128, 32, W], BF16, tag="prod", name="prod")
                xw = _xwin(pt, YM + 32 * half + ky + sy, XM + kx + sx, 32)
                nc.vector.tensor_tensor(prod[:], ut[:], xw, ALU.mult)
                first = (k == 0 and s == 0)
                last = (k == 8 and s == 8)
                for c in range(2):
                    for q in range(2):
                        nc.tensor.matmul(
                            banks[c][:, q * 512:(q + 1) * 512], dcn[:, k, :],
                            prod[:, c * 16 + q * 8:c * 16 + q * 8 + 8, :],
                            start=first, stop=last)
            pop_units()
        for c in range(2):
            nc.scalar.activation(
                xr2d[:, half * 2048 + c * 1024:half * 2048 + (c + 1) * 1024],
                banks[c][:], AF.Relu, bias=_w1(wt, "dcn_b_pk"))
    while pending:
        pending.pop(0)()


def _ph3(tc, nc, p, pt, xr2d, xin, yout, ph3p):
    s0 = 2 * p
    for hf in range(2):
        cols = slice(hf * 2048, (hf + 1) * 2048)
        gxr = ph3p.tile([128, 2048], F32, tag="gxr", name=f"gxr{p}{hf}")
        for sl in range(2):
            nc.sync.dma_start(gxr[64 * sl:64 * sl + 64, :],
                              xin[s0 + sl][:, cols])
        out2 = ph3p.tile([128, 2048], BF16, tag="o2", name=f"o2{p}{hf}")
        nc.vector.tensor_tensor(
            out2[:], xr2d[:, cols],
            pt["ca"][:].broadcast_to([128, 2048]), ALU.mult)
        nc.vector.tensor_tensor(out2[:], out2[:], pt["out0"][:, cols],
                                ALU.add)
        nc.scalar.activation(out2[:], out2[:], AF.Sigmoid)
        nc.vector.tensor_tensor(gxr[:], gxr[:], out2[:], ALU.mult)
        for sl in range(2):
            nc.sync.dma_start(yout[s0 + sl][:, cols],
                              gxr[64 * sl:64 * sl + 64, :])


# ---------------------------------------------------------------------------
# entry point
# ---------------------------------------------------------------------------

_CACHE = {}


def _prep_all(inputs):
    x = np.asarray(inputs["x"], np.float32)
    assert x.shape == (2, 1024, 64, 64)
    x_slices = np.ascontiguousarray(x.reshape(32, 64, HW))
    wd = _host_prep(inputs)
    off = _host_offsets(x_slices, wd)
    plans_all = _plan_rare(off)          # 16 pairs (32 slices)
    return x_slices, wd, plans_all


USE_RARE = False          # rare ring corrections (cross-core union program)


def kernel(**inputs):
    x_slices, wd, plans_all = _prep_all(inputs)
    x16 = x_slices.astype(ml_dtypes.bfloat16)

    # One SPMD program for all cores: plan structure must be core-uniform,
    # so use the empty plan (relu-clipped hats only; rel err ~6e-3).
    plans = [[], []]

    key = (repr(plans),)
    if key not in _CACHE:
        _CACHE[key] = (build_nc(wd, plans), _build_blobs(wd, plans))
    nc, (b1, b2, _, _, _, _) = _CACHE[key]
    in_maps = []
    for core in range(NCORES):
        in_maps.append({
            "xin": np.ascontiguousarray(
                x_slices[core * NSLICES:(core + 1) * NSLICES]),
            "xin16": np.ascontiguousarray(
                x16[core * NSLICES:(core + 1) * NSLICES]),
            "wblob": b1,
            "wblob16": b2,
        })
    res = run_bass_kernel_spmd(nc, in_maps, list(range(NCORES))).results
    out = np.empty((32, 64, HW), np.float32)
    for core in range(NCORES):
        out[core * NSLICES:(core + 1) * NSLICES] = res[core]["yout"]
    return out.reshape(2, 1024, 64, 64)


if __name__ == "__main__":
    import reference
    inputs = {k: np.asarray(v) for k, v in reference.setup_inputs().items()}
    got = kernel(**inputs)
    print("kernel output:", got.shape, got.dtype)


# revision 17
# speedup vs baseline: 2.9794x; 2.9794x over previous
"""Trainium2 Bass kernel for nn_MDA_4183298146862 (MDA dense_cnn module).

[2,1024,64,64] -> 32 group slices [64ch,64,64]; 4 per core (2 pairs packed
2-per-128-partitions).  All matmuls bf16 with slice-block-diagonal
stationaries so one matmul serves both packed slices.

DCNv2 via the commuted hat decomposition: for tap k and stencil shift
s=(sy,sx) in the 3x3 core, the per-pixel weight u_{k,s} = m_k*hy(sy)*hx(sx)
multiplies the *input* field X=x2n (shifted by t_k+s); the sums over all 81
(k,s) terms and over channels run inside PSUM accumulation:
    out[o,p] = sum_{k,s,c} Wdcn[o,c,k] * u_{k,s}(p) * X[c, p+t_k+s]
DVE only does one elementwise multiply per term.  u fields are built on
<=81-partition tiles with permutation matmuls (no small-DMA storms),
written to DRAM scratch, and broadcast-replicated across the 64 channel
partitions by DMA at consumption time.  Hat weights relu-clip for |d|>1;
the few high-weight |d|>1 pixel clusters (host-planned, weight>0.02) get
exact extra ring terms accumulated the same way on clipped row ranges.
"""

import numpy as np
import ml_dtypes
from contextlib import ExitStack

import concourse.bass as bass
import concourse.bacc as bacc
import concourse.tile as tile
import concourse.mybir as mybir
from concourse.bass_utils import run_bass_kernel_spmd

F32 = mybir.dt.float32
BF16 = mybir.dt.bfloat16
AF = mybir.ActivationFunctionType
ALU = mybir.AluOpType
AX = mybir.AxisListType

EPS32 = 1.1920929e-07
BN_EPS = 1e-5
GN_EPS = 1e-5
H = W = 64
HW = H * W
NCORES = 8
NSLICES = 4
PAIRS = 2
YCH = 8
NCH = H // YCH
RARE_TH = 0.02
MERGED_BCAST = True          # one stride-0-source DMA vs two partition_broadcasts
BCAST_ENGINES = "ssag"       # rotation: s=sync a=scalar g=gpsimd

YM = 3
XM = 4
SLAB_H = YM + H + 3      # 70
SLAB_W = XM + W + 4      # 72


# ---------------------------------------------------------------------------
# host-side preprocessing
# ---------------------------------------------------------------------------

def _blkdiag(a):
    """[k,m] -> [2k, 2m] block diagonal."""
    k, m = a.shape
    out = np.zeros((2 * k, 2 * m), np.float32)
    out[:k, :m] = a
    out[k:, m:] = a
    return out


def _host_prep(inputs):
    f = np.float32
    g = lambda n: np.asarray(inputs[n], f)
    w = {}
    bn_s = g("inv_bn_g") / np.sqrt(1.0 + BN_EPS)

    # legacy layouts for the host offset simulation
    w["c3_lhsT"] = np.ascontiguousarray(
        g("c3_w").reshape(64, 64, 9).transpose(1, 2, 0))
    w["c3_b"] = g("c3_b").reshape(64, 1)
    w["gn_g"] = g("gn_g").reshape(64, 1)
    w["gn_b"] = g("gn_b").reshape(64, 1)
    perm = list(range(0, 18, 2)) + list(range(1, 18, 2)) + list(range(18, 27))
    w["off_lhsT"] = np.ascontiguousarray(
        g("off_w")[perm].reshape(27, 64, 9).transpose(1, 2, 0))
    w["off_b"] = g("off_b")[perm].reshape(27, 1)

    # bf16 block-diagonal stationaries
    w["invred_blk"] = _blkdiag(g("inv_reduce_w").T)               # [128,32]
    w["span_blk"] = _blkdiag(g("inv_span_w").T)                   # [32,8]
    rep16 = np.zeros((4, 64), f)
    for i in range(4):
        rep16[i, i * 16:(i + 1) * 16] = 1.0
    w["rep16_blk"] = _blkdiag(rep16)                              # [8,128]
    w["red_blk"] = _blkdiag(g("red_w").T)                         # [128,64]
    w["res_blkf"] = _blkdiag((g("res_w") / 64.0).T)               # [64,128]
    w["fc1_blkf"] = _blkdiag(g("fc1_w").T)                        # [128,32]
    w["fc2_blkf"] = _blkdiag(g("fc2_w").T)                        # [32,128]
    c3T = w["c3_lhsT"]                                            # [64,9,64]
    w["c3_blk"] = np.stack([_blkdiag(c3T[:, t, :]) for t in range(9)],
                           1).reshape(128, 9 * 128)
    offT = w["off_lhsT"]                                          # [64,9,27]
    colmap = np.zeros(54, np.int64)        # old blockdiag col -> new row
    for sl in range(2):
        for t in range(9):
            colmap[sl * 27 + 0 + t] = 18 + sl * 18 + t      # dy
            colmap[sl * 27 + 9 + t] = 27 + sl * 18 + t      # dx
            colmap[sl * 27 + 18 + t] = sl * 9 + t           # mask
    ob = np.stack([_blkdiag(offT[:, t, :]) for t in range(9)], 1)  # [128,9,54]
    ob2 = np.zeros_like(ob)
    ob2[:, :, colmap] = ob
    w["off_blk"] = ob2.reshape(128, 9 * 54)
    dcnT = np.ascontiguousarray(
        g("dcn_w").reshape(64, 64, 9).transpose(1, 2, 0))         # [64,9,64]
    w["dcn_blk"] = np.stack([_blkdiag(dcnT[:, t, :]) for t in range(9)],
                            1).reshape(128, 9 * 128)

    # permutation stationaries for hat-field construction
    def prep3(kind):
        p = np.zeros((54, 54), f)
        for sl in range(2):
            for grp in range(3):
                for t in range(9):
                    if kind == "y":
                        src = 18 + sl * 18 + t
                    elif kind == "x":
                        src = 27 + sl * 18 + t
                    else:
                        src = sl * 9 + t
                    p[src, sl * 27 + grp * 9 + t] = 1.0
        return p
    w["P_repy"] = prep3("y")
    w["P_repx"] = prep3("x")
    w["P_repm"] = prep3("m")
    for sl in range(2):
        pa = np.zeros((54, 81), f)
        px = np.zeros((54, 81), f)
        for sy in range(3):
            for sx in range(3):
                for t in range(9):
                    r = (sy * 3 + sx) * 9 + t
                    pa[sl * 27 + sy * 9 + t, r] = 1.0
                    px[sl * 27 + sx * 9 + t, r] = 1.0
        w[f"P_ayS{sl}"] = pa
        w[f"P_axS{sl}"] = px

    # f32 bias/const columns
    w["inv_scale_pk"] = np.tile(bn_s, 2).reshape(32, 1)
    w["inv_bias_pk"] = np.tile(bn_s * g("inv_reduce_b") + g("inv_bn_b"),
                               2).reshape(32, 1)
    w["span_b_pk"] = np.tile(g("inv_span_b"), 2).reshape(8, 1)
    red_b = g("red_b") + EPS32 * g("red_w").sum(1)
    w["red_b_pk"] = np.tile(red_b, 2).reshape(64, 1)
    w["res_b_pk"] = np.tile(g("res_b"), 2).reshape(128, 1)
    w["c3_b_pk"] = np.tile(g("c3_b"), 2).reshape(128, 1)
    w["gn_g_pk"] = np.tile(g("gn_g"), 2).reshape(128, 1)
    w["gn_b_pk"] = np.tile(g("gn_b"), 2).reshape(128, 1)
    ob_b = np.zeros(54, f)
    ob_b[colmap] = np.tile(g("off_b")[perm], 2)
    w["off_b_pk"] = ob_b.reshape(54, 1)
    w["dcn_b_pk"] = np.tile(g("dcn_b"), 2).reshape(128, 1)
    sy_b = np.concatenate([np.repeat([1.0, 0.0, -1.0], 9)] * 2)
    w["sy_bias54"] = sy_b.reshape(54, 1)
    w["ones54"] = np.ones((54, 1), f)
    rb = np.zeros(54, f)
    for sl in range(2):
        rb[sl * 27 + 0:sl * 27 + 9] = -2.0    # sign +2 rows: |d - 2|
        rb[sl * 27 + 9:sl * 27 + 18] = 2.0    # sign -2 rows: |d + 2|
    w["rare_b54"] = rb.reshape(54, 1)
    return w


def _host_offsets(x_slices, wd):
    """Offset fields [S, 27, H, W] on host (fp32 sim of device pipeline)."""
    S = x_slices.shape[0]
    xs = x_slices.reshape(S, 64, H, W).astype(np.float32)

    def conv3x3(inp, lhsT, nout):
        pad = np.zeros((S, 64, H + 2, W + 2), np.float32)
        pad[:, :, 1:-1, 1:-1] = inp
        out = np.zeros((S, nout, H, W), np.float32)
        for t in range(9):
            ty, tx = t // 3, t % 3
            win = pad[:, :, ty:ty + H, tx:tx + W]
            out += np.einsum("co,schw->sohw", lhsT[:, t, :], win,
                             optimize=True)
        return out

    xc3 = conv3x3(xs, wd["c3_lhsT"], 64) + wd["c3_b"].reshape(1, 64, 1, 1)
    mu = xc3.mean(axis=(2, 3), keepdims=True)
    var = xc3.var(axis=(2, 3), keepdims=True)
    x2n = ((xc3 - mu) / np.sqrt(var + GN_EPS)
           * wd["gn_g"].reshape(1, 64, 1, 1) + wd["gn_b"].reshape(1, 64, 1, 1))
    return conv3x3(x2n, wd["off_lhsT"], 27) + wd["off_b"].reshape(1, 27, 1, 1)


def _plan_rare(off_fields, th=RARE_TH):
    """Per pair: list of (sl, k, sy, sx, ya, yb, akind, bkind) for ring terms
    with max hat-product weight > th.  akind: ('core', sy) or ('rare', sign);
    bkind likewise for x."""
    S = off_fields.shape[0]
    sig = 1.0 / (1.0 + np.exp(-off_fields[:, 18:27]))
    plans = [[] for _ in range(S // 2)]
    for s in range(S):
        pair = s // 2
        sl = s % 2
        for k in range(9):
            dy = off_fields[s, k]
            dx = off_fields[s, 9 + k]
            m = sig[s, k]
            for sy in (-2, -1, 0, 1, 2):
                hy = np.maximum(0.0, 1.0 - np.abs(dy - sy))
                for sx in (-2, -1, 0, 1, 2):
                    if abs(sy) <= 1 and abs(sx) <= 1:
                        continue
                    hx = np.maximum(0.0, 1.0 - np.abs(dx - sx))
                    wf = hy * hx * m
                    if wf.max() <= th:
                        continue
                    rows = np.nonzero((wf > th).any(axis=1))[0]
                    ya, yb = int(rows[0]), int(rows[-1] + 1)
                    ak = ('core', sy) if abs(sy) <= 1 else ('rare', sy // 2)
                    bk = ('core', sx) if abs(sx) <= 1 else ('rare', sx // 2)
                    plans[pair].append((sl, k, sy, sx, ya, yb, ak, bk))
    return plans


# ---------------------------------------------------------------------------
# weight blobs
# ---------------------------------------------------------------------------

_F32_SPEC = [
    ("inv_scale_pk", 1, 32), ("inv_bias_pk", 1, 32), ("span_b_pk", 1, 8),
    ("red_b_pk", 1, 64), ("res_b_pk", 1, 128), ("c3_b_pk", 1, 128),
    ("gn_g_pk", 1, 128), ("gn_b_pk", 1, 128), ("off_b_pk", 1, 54),
    ("dcn_b_pk", 1, 128), ("sy_bias54", 1, 54), ("ones54", 1, 54),
    ("rare_b54", 1, 54), ("res_blkf", 128, 64), ("fc1_blkf", 32, 128),
    ("fc2_blkf", 128, 32),
]

_BF16_SPEC = [
    ("invred_blk", 32, 128), ("span_blk", 8, 32), ("rep16_blk", 128, 8),
    ("red_blk", 64, 128), ("c3_blk", 9 * 128, 128), ("off_blk", 9 * 54, 128),
    ("dcn_blk", 9 * 128, 128), ("P_repy", 54, 54), ("P_repx", 54, 54),
    ("P_repm", 54, 54), ("P_ayS0", 81, 54), ("P_ayS1", 81, 54),
    ("P_axS0", 81, 54), ("P_axS1", 81, 54),
]


def _rare_pmats(plans):
    """Selector stationaries for rare-u row stacks, per pair.
    A-side sources ay54 (core) / rf_y (rare); B-side hx54 / rf_x.
    rf tiles: rows sl*27 + (0 if sign>0 else 9) + t."""
    mats = {}
    for p, entries in enumerate(plans):
        E = max(1, len(entries))
        pca = np.zeros((54, E), np.float32)
        pra = np.zeros((54, E), np.float32)
        pcb = np.zeros((54, E), np.float32)
        prb = np.zeros((54, E), np.float32)
        for e, (sl, k, sy, sx, ya, yb, ak, bk) in enumerate(entries):
            if ak[0] == 'core':
                pca[sl * 27 + (ak[1] + 1) * 9 + k, e] = 1.0
            else:
                pra[sl * 27 + (0 if ak[1] > 0 else 9) + k, e] = 1.0
            if bk[0] == 'core':
                pcb[sl * 27 + (bk[1] + 1) * 9 + k, e] = 1.0
            else:
                prb[sl * 27 + (0 if bk[1] > 0 else 9) + k, e] = 1.0
        mats[f"P_rcA{p}"] = pca
        mats[f"P_rrA{p}"] = pra
        mats[f"P_rcB{p}"] = pcb
        mats[f"P_rrB{p}"] = prb
    return mats


def _build_specs(plans):
    f32_spec = list(_F32_SPEC)
    bf16_spec = list(_BF16_SPEC)
    for p in range(PAIRS):
        E = max(1, len(plans[p]))
        for nm in (f"P_rcA{p}", f"P_rrA{p}", f"P_rcB{p}", f"P_rrB{p}"):
            bf16_spec.append((nm, E, 54))
    return f32_spec, bf16_spec


def _cols(spec):
    cols = {}
    o = 0
    for name, ncols, kdim in spec:
        cols[name] = (o, ncols, kdim)
        o += ncols
    return cols


def _build_blobs(wd, plans):
    f32_spec, bf16_spec = _build_specs(plans)
    rmats = _rare_pmats(plans)
    c1 = _cols(f32_spec)
    n1 = sum(n for _, n, _ in f32_spec)
    b1 = np.zeros((128, n1), np.float32)
    for name, (o, ncols, kdim) in c1.items():
        b1[0:kdim, o:o + ncols] = wd[name].reshape(kdim, ncols)
    c2 = _cols(bf16_spec)
    n2 = sum(n for _, n, _ in bf16_spec)
    b2 = np.zeros((128, n2), np.float32)
    for name, (o, ncols, kdim) in c2.items():
        arr = rmats[name] if name in rmats else wd[name]
        b2[0:kdim, o:o + ncols] = arr.reshape(kdim, ncols)
    return b1, b2.astype(ml_dtypes.bfloat16), c1, c2, n1, n2


# ---------------------------------------------------------------------------
# bass program
# ---------------------------------------------------------------------------

def build_nc(wd, plans, repeat=1, debug=False):
    b1, b2, c1, c2, n1, n2 = _build_blobs(wd, plans)
    nc = bacc.Bacc("TRN2", target_bir_lowering=False, debug=debug)
    xin = nc.dram_tensor("xin", [NSLICES, 64, HW], F32,
                         kind="ExternalInput").ap()
    xin16 = nc.dram_tensor("xin16", [NSLICES, 64, HW], BF16,
                           kind="ExternalInput").ap()
    yout = nc.dram_tensor("yout", [NSLICES, 64, HW], F32,
                          kind="ExternalOutput").ap()
    wb1 = nc.dram_tensor("wblob", [128, n1], F32, kind="ExternalInput").ap()
    wb2 = nc.dram_tensor("wblob16", [128, n2], BF16,
                         kind="ExternalInput").ap()
    uscr = [nc.dram_tensor(f"uscr{p}", [2, 81, HW], BF16).ap()
            for p in range(PAIRS)]
    rscr = [nc.dram_tensor(f"rscr{p}", [max(1, len(plans[p])), HW], BF16).ap()
            for p in range(PAIRS)]

    st = {"bc": 0}   # broadcast engine rotation

    with tile.TileContext(nc) as tc:
        with ExitStack() as ctx:
            tp = lambda **kw: ctx.enter_context(tc.tile_pool(**kw))
            consts = tp(name="consts", bufs=1)
            pers = tp(name="pers", bufs=1)
            big = tp(name="big", bufs=1)
            t128 = tp(name="t128", bufs=2)      # xc3 / xr2d rotation
            chk = tp(name="chk", bufs=2)        # conv chunk stages
            hat = tp(name="hat", bufs=2)        # hat-build 512-col tiles
            smalls = tp(name="smalls", bufs=2)
            ph2p = tp(name="ph2p", bufs=3)
            utp = tp(name="utp", bufs=6)
            offp = tp(name="offp", bufs=2)
            ph3p = tp(name="ph3p", bufs=1)
            psum1 = tp(name="psum1", bufs=4, space="PSUM")
            psamp = tp(name="psamp", bufs=1, space="PSUM")

            blob1 = consts.tile([128, n1], F32, tag="b1", name="b1")
            blob2 = consts.tile([128, n2], BF16, tag="b2", name="b2")
            nc.sync.dma_start(blob1[:], wb1[:])
            nc.sync.dma_start(blob2[:], wb2[:])
            ccols = {}
            for v in (GN_EPS,):
                t = consts.tile([128, 1], F32, tag=f"cc{v}", name=f"cc{v}")
                nc.gpsimd.memset(t[:], float(v))
                ccols[float(v)] = t

            wt = {"b1": blob1, "b2": blob2, "c1": c1, "c2": c2,
                  "ccols": ccols}

            tc.strict_bb_all_engine_barrier()

            pair_t = []
            for p in range(PAIRS):
                pair_t.append({
                    "x2ne": pers.tile([128, SLAB_H, SLAB_W], BF16,
                                      tag=f"x2ne{p}", name=f"x2ne{p}"),
                    "x2no": pers.tile([128, SLAB_H, SLAB_W - 1], BF16,
                                      tag=f"x2no{p}", name=f"x2no{p}"),
                    "out0": pers.tile([128, HW], BF16, tag=f"out0{p}",
                                      name=f"out0{p}"),
                    "ca": pers.tile([128, 1], F32, tag=f"ca{p}",
                                    name=f"ca{p}"),
                })

            gxp = tp(name="gxp", bufs=2)
            for rep in range(repeat):
                args = (tc, nc, xin16, wt, plans, uscr, rscr,
                        big, t128, chk, hat, smalls, psum1, st, gxp, offp,
                        utp)
                gx0 = _ph1_load(nc, 0, pair_t[0], xin16, gxp)
                gx1 = _ph1_load(nc, 1, pair_t[1], xin16, gxp)
                u0 = _ph1_dcn_units(0, pair_t[0], gx0, *args)
                u1 = _ph1_dcn_units(1, pair_t[1], gx1, *args)
                for f in u0:
                    f()
                r0 = _ph1_rest_units(0, pair_t[0], gx0, *args)
                r1 = _ph1_rest_units(1, pair_t[1], gx1, *args)
                xr2d0 = t128.tile([128, HW], BF16, tag="t128", name="xr2d0")
                _ph2(tc, nc, 0, pair_t[0], xr2d0, wt, plans, uscr, rscr,
                     ph2p, psamp, st, interleave=u1 + r0, utp=utp)
                _ph3(tc, nc, 0, pair_t[0], xr2d0, xin, yout, ph3p)
                xr2d1 = t128.tile([128, HW], BF16, tag="t128", name="xr2d1")
                _ph2(tc, nc, 1, pair_t[1], xr2d1, wt, plans, uscr, rscr,
                     ph2p, psamp, st, interleave=r1, utp=utp)
                _ph3(tc, nc, 1, pair_t[1], xr2d1, xin, yout, ph3p)
    nc.compile()
    return nc


def _w1(wt, name, nparts=None):
    o, ncols, kdim = wt["c1"][name]
    return wt["b1"][0:(nparts or kdim), o:o + ncols]


def _w2(wt, name):
    o, ncols, kdim = wt["c2"][name]
    ap = wt["b2"][0:kdim, o:o + ncols]
    if ncols > 160:
        ap = ap.rearrange("k (t m) -> k t m", t=9)
    return ap


_ENG = {"s": "sync", "a": "scalar", "g": "gpsimd"}


def _bcast_dma(nc, st, out_ap, in_ap):
    rot = BCAST_ENGINES
    eng = getattr(nc, _ENG[rot[st["bc"] % len(rot)]])
    st["bc"] += 1
    eng.dma_start(out_ap, in_ap)


def _zero_margins(nc, slab, wdt):
    nc.gpsimd.memset(slab[:, 0:YM, :], 0.0)
    nc.gpsimd.memset(slab[:, YM + H:SLAB_H, :], 0.0)
    nc.gpsimd.memset(slab[:, YM:YM + H, 0:XM], 0.0)
    nc.gpsimd.memset(slab[:, YM:YM + H, XM + W:wdt], 0.0)


def _ph1_load(nc, p, pt, xin16, gxp):
    s0 = 2 * p
    gx = gxp.tile([128, SLAB_H, SLAB_W], BF16, tag="gx", name=f"gx{p}")
    _zero_margins(nc, gx, SLAB_W)
    _zero_margins(nc, pt["x2ne"], SLAB_W)
    for sl in range(2):
        nc.sync.dma_start(gx[64 * sl:64 * sl + 64, YM:YM + H, XM:XM + W],
                          xin16[s0 + sl].rearrange("c (h w) -> c h w", w=W))
    return gx


def _gxw(gx, ch, dy=0, dx=0):
    return gx[:, YM + ch * YCH + dy:YM + ch * YCH + dy + YCH,
              XM + dx:XM + dx + W]


def _ph1_dcn_units(p, pt, gx, tc, nc, xin16, wt, plans, uscr, rscr,
                   big, t128, chk, hat, smalls, psum1, st, gxp, offp, utp):
    """Closures (in order): 8x c3-chunk, GN, 8x off-chunk, 8x hat-chunk."""
    entries = plans[p]
    x2ne = pt["x2ne"]
    xc3 = t128.tile([128, HW], BF16, tag="t128", name=f"xc3{p}")
    offpk = offp.tile([54, HW], BF16, tag="off", name=f"off{p}")
    sumc = smalls.tile([128, NCH], F32, tag="sumc", name=f"sumc{p}")
    sqc = smalls.tile([128, NCH], F32, tag="sqc", name=f"sqc{p}")
    units = []

    def c3_chunk(ch):
        def f():
            cols = slice(ch * 512, (ch + 1) * 512)
            pc = psum1.tile([128, 512], F32, tag="pc", name="pcE")
            for t in range(9):
                ty, tx = t // 3, t % 3
                nc.tensor.matmul(pc[:], _w2(wt, "c3_blk")[:, t, :],
                                 _gxw(gx, ch, ty - 1, tx - 1),
                                 start=(t == 0), stop=(t == 8))
            nc.scalar.activation(xc3[:, cols], pc[:], AF.Identity,
                                 bias=_w1(wt, "c3_b_pk"),
                                 accum_out=sumc[:, ch:ch + 1])
            scr = chk.tile([128, 512], BF16, tag="scr", name="scr")
            nc.scalar.activation(scr[:], xc3[:, cols], AF.Square,
                                 accum_out=sqc[:, ch:ch + 1])
        return f

    def gn_apply():
        mu = smalls.tile([128, 1], F32, tag="mu", name="mu")
        nc.vector.tensor_reduce(mu[:], sumc[:], AX.X, ALU.add)
        nc.scalar.activation(mu[:], mu[:], AF.Identity, scale=1.0 / HW)
        vr = smalls.tile([128, 1], F32, tag="vr", name="vr")
        nc.vector.tensor_reduce(vr[:], sqc[:], AX.X, ALU.add)
        nc.scalar.activation(vr[:], vr[:], AF.Identity, scale=1.0 / HW)
        ms = smalls.tile([128, 1], F32, tag="ms", name="ms")
        nc.gpsimd.tensor_tensor(ms[:], mu[:], mu[:], ALU.mult)
        nc.gpsimd.tensor_sub(vr[:], vr[:], ms[:])
        nc.scalar.activation(vr[:], vr[:], AF.Sqrt,
                             bias=wt["ccols"][GN_EPS][0:128, :])
        istd = smalls.tile([128, 1], F32, tag="istd", name="istd")
        nc.vector.reciprocal(istd[:], vr[:])
        sc = smalls.tile([128, 1], F32, tag="sc", name="sc")
        nc.gpsimd.tensor_tensor(sc[:], istd[:], _w1(wt, "gn_g_pk"), ALU.mult)
        bi = smalls.tile([128, 1], F32, tag="bi", name="bi")
        nc.gpsimd.tensor_tensor(bi[:], mu[:], sc[:], ALU.mult)
        nc.gpsimd.tensor_sub(bi[:], _w1(wt, "gn_b_pk"), bi[:])
        nc.scalar.activation(x2ne[:, YM:YM + H, XM:XM + W],
                             xc3[:].rearrange("c (h w) -> c h w", w=W),
                             AF.Identity, bias=bi[:], scale=sc[:])

    def x2no_copy():
        nc.scalar.activation(pt["x2no"][:], x2ne[:, :, 1:SLAB_W],
                             AF.Identity)

    def x2w(ch, dy=0, dx=0):
        return x2ne[:, YM + ch * YCH + dy:YM + ch * YCH + dy + YCH,
                    XM + dx:XM + dx + W]

    def off_chunk(ch):
        def f():
            pc = psum1.tile([128, 512], F32, tag="pc", name="pcI")
            for t in range(9):
                ty, tx = t // 3, t % 3
                nc.tensor.matmul(pc[0:54, :], _w2(wt, "off_blk")[:, t, :],
                                 x2w(ch, ty - 1, tx - 1),
                                 start=(t == 0), stop=(t == 8))
            cols = slice(ch * 512, (ch + 1) * 512)
            nc.scalar.activation(offpk[:, cols], pc[0:54, :], AF.Identity,
                                 bias=_w1(wt, "off_b_pk"))
            nc.scalar.activation(offpk[0:18, cols], offpk[0:18, cols],
                                 AF.Sigmoid)
        return f

    have_rare = len(entries) > 0

    def hat_chunk(ch):
        def f():
            cols = slice(ch * 512, (ch + 1) * 512)
            opk = offpk[:, cols]
            pd = psum1.tile([128, 512], F32, tag="pc", name="pcJ")
            nc.tensor.matmul(pd[0:54, :], _w2(wt, "P_repy"), opk,
                             start=True, stop=True)
            t54 = hat.tile([54, 512], BF16, tag="h54", name="t54")
            nc.scalar.activation(t54[:], pd[0:54, :], AF.Abs,
                                 bias=_w1(wt, "sy_bias54"))
            hy54 = hat.tile([54, 512], BF16, tag="h54", name="hy54")
            nc.scalar.activation(hy54[:], t54[:], AF.Relu,
                                 bias=_w1(wt, "ones54"), scale=-1.0)
            pm = psum1.tile([128, 512], F32, tag="pc", name="pcK")
            nc.tensor.matmul(pm[0:54, :], _w2(wt, "P_repm"), opk,
                             start=True, stop=True)
            ay54 = hat.tile([54, 512], BF16, tag="ay", name="ay54")
            nc.vector.tensor_tensor(ay54[:], hy54[:], pm[0:54, :], ALU.mult)
            pdx = psum1.tile([128, 512], F32, tag="pc", name="pcL")
            nc.tensor.matmul(pdx[0:54, :], _w2(wt, "P_repx"), opk,
                             start=True, stop=True)
            t54x = hat.tile([54, 512], BF16, tag="h54", name="t54x")
            nc.scalar.activation(t54x[:], pdx[0:54, :], AF.Abs,
                                 bias=_w1(wt, "sy_bias54"))
            hx54 = hat.tile([54, 512], BF16, tag="hx", name="hx54")
            nc.scalar.activation(hx54[:], t54x[:], AF.Relu,
                                 bias=_w1(wt, "ones54"), scale=-1.0)
            if have_rare:
                rfy = hat.tile([54, 512], BF16, tag="rfy", name="rfy")
                nc.scalar.activation(rfy[0:45, :], pd[0:45, :], AF.Abs,
                                     bias=_w1(wt, "rare_b54", 45))
                nc.scalar.activation(rfy[0:45, :], rfy[0:45, :], AF.Relu,
                                     bias=_w1(wt, "ones54", 45), scale=-1.0)
                nc.vector.tensor_tensor(rfy[0:45, :], rfy[0:45, :],
                                        pm[0:45, :], ALU.mult)
                rfx = hat.tile([54, 512], BF16, tag="rfx", name="rfx")
                nc.scalar.activation(rfx[0:45, :], pdx[0:45, :], AF.Abs,
                                     bias=_w1(wt, "rare_b54", 45))
                nc.scalar.activation(rfx[0:45, :], rfx[0:45, :], AF.Relu,
                                     bias=_w1(wt, "ones54", 45), scale=-1.0)
                E = len(entries)
                pA = psum1.tile([128, 512], F32, tag="pc", name="pcM")
                nc.tensor.matmul(pA[0:E, :], _w2(wt, f"P_rcA{p}"), ay54[:],
                                 start=True, stop=False)
                nc.tensor.matmul(pA[0:E, :], _w2(wt, f"P_rrA{p}"), rfy[:],
                                 start=False, stop=True)
                pB = psum1.tile([128, 512], F32, tag="pc", name="pcN")
                nc.tensor.matmul(pB[0:E, :], _w2(wt, f"P_rcB{p}"), hx54[:],
                                 start=True, stop=False)
                nc.tensor.matmul(pB[0:E, :], _w2(wt, f"P_rrB{p}"), rfx[:],
                                 start=False, stop=True)
                bE = hat.tile([54, 512], BF16, tag="bE", name="bE")
                nc.scalar.activation(bE[0:E, :], pB[0:E, :], AF.Identity)
                ur = hat.tile([54, 512], BF16, tag="ur", name="ur")
                nc.vector.tensor_tensor(ur[0:E, :], pA[0:E, :], bE[0:E, :],
                                        ALU.mult)
                nc.gpsimd.dma_start(rscr[p][:, cols], ur[0:E, :])
            for sl in range(2):
                pa = psum1.tile([128, 512], F32, tag="pc", name="pcO")
                nc.tensor.matmul(pa[0:81, :], _w2(wt, f"P_ayS{sl}"),
                                 ay54[:], start=True, stop=True)
                px = psum1.tile([128, 512], F32, tag="pc", name="pcP")
                nc.tensor.matmul(px[0:81, :], _w2(wt, f"P_axS{sl}"),
                                 hx54[:], start=True, stop=True)
                axc = hat.tile([81, 512], BF16, tag="axc", name="axc")
                nc.scalar.activation(axc[:], px[0:81, :], AF.Identity)
                u81 = hat.tile([81, 512], BF16, tag="u81", name="u81")
                nc.vector.tensor_tensor(u81[:], pa[0:81, :], axc[:],
                                        ALU.mult)
                nc.gpsimd.dma_start(uscr[p][sl, :, cols], u81[:])
        return f

    for ch in range(NCH):
        units.append(c3_chunk(ch))
    units.append(gn_apply)
    for ch in range(NCH):
        units.append(off_chunk(ch))
        units.append(hat_chunk(ch))
        if ch == 0:
            units.append(x2no_copy)
    return units


def _ph1_rest_units(p, pt, gx, tc, nc, xin16, wt, plans, uscr, rscr,
                    big, t128, chk, hat, smalls, psum1, st, gxp, offp, utp):
    """Involution + coordinate attention + channel attention closures."""
    xr = big.tile([64, HW], BF16, tag="xr", name=f"xr{p}")
    units = []

    def inv_chunk(ch):
        def f():
            cols = slice(ch * 512, (ch + 1) * 512)
            pc = psum1.tile([128, 512], F32, tag="pc", name="pcA")
            nc.tensor.matmul(pc[0:32, :], _w2(wt, "invred_blk"),
                             _gxw(gx, ch), start=True, stop=True)
            r_ch = chk.tile([32, 512], BF16, tag="r", name="r")
            nc.scalar.activation(r_ch[:], pc[0:32, :], AF.Relu,
                                 bias=_w1(wt, "inv_bias_pk"),
                                 scale=_w1(wt, "inv_scale_pk"))
            pc = psum1.tile([128, 512], F32, tag="pc", name="pcB")
            nc.tensor.matmul(pc[0:8, :], _w2(wt, "span_blk"), r_ch[:],
                             start=True, stop=True)
            wm_ch = chk.tile([8, 512], BF16, tag="wm", name="wm")
            nc.scalar.activation(wm_ch[:], pc[0:8, :], AF.Identity,
                                 bias=_w1(wt, "span_b_pk"))
            pc = psum1.tile([128, 512], F32, tag="pc", name="pcC")
            nc.tensor.matmul(pc[:], _w2(wt, "rep16_blk"), wm_ch[:],
                             start=True, stop=True)
            xr1_ch = chk.tile([128, 512], BF16, tag="xr1", name="xr1")
            nc.vector.tensor_tensor(
                xr1_ch[:].rearrange("c (a b) -> c a b", b=W),
                pc[:].rearrange("c (a b) -> c a b", b=W), _gxw(gx, ch),
                ALU.mult)
            pc = psum1.tile([128, 512], F32, tag="pc", name="pcD")
            nc.tensor.matmul(pc[0:64, :], _w2(wt, "red_blk"), xr1_ch[:],
                             start=True, stop=True)
            nc.scalar.activation(xr[:, cols], pc[0:64, :], AF.Identity,
                                 bias=_w1(wt, "red_b_pk"))
        return f

    def coord():
        cat = smalls.tile([64, 128], F32, tag="cat", name="cat")
        xr3 = xr[:].rearrange("c (h w) -> c h w", w=W)
        nc.vector.tensor_reduce(cat[:, 0:64], xr3, AX.X, ALU.add)
        nc.vector.tensor_reduce(cat[:, 64:128], xr3.transpose([0, 2, 1]),
                                AX.X, ALU.add)
        pc = psum1.tile([128, 512], F32, tag="pc", name="pcF")
        nc.tensor.matmul(pc[:, 0:128], _w1(wt, "res_blkf"), cat[:],
                         start=True, stop=True)
        hw_pk = smalls.tile([128, 128], BF16, tag="hw", name="hw")
        nc.scalar.activation(hw_pk[:], pc[:, 0:128], AF.Sigmoid,
                             bias=_w1(wt, "res_b_pk"))
        sh_pk = smalls.tile([128, 64], BF16, tag="sh", name="sh")
        nc.scalar.activation(sh_pk[:], hw_pk[:, 0:64], AF.Sigmoid)
        nc.vector.tensor_tensor(
            pt["out0"][:].rearrange("c (h w) -> c h w", w=W),
            gx[:, YM:YM + H, XM:XM + W],
            sh_pk[:, :, None].broadcast_to([128, 64, 64]), ALU.mult)

    def chattn():
        am = smalls.tile([128, 2], F32, tag="am", name="am")
        o0f = pt["out0"][:]
        nc.vector.tensor_reduce(am[:, 0:1], o0f, AX.X, ALU.add)
        nc.vector.tensor_reduce(am[:, 1:2], o0f, AX.X, ALU.max)
        nc.scalar.activation(am[:, 0:1], am[:, 0:1], AF.Identity,
                             scale=1.0 / HW)
        pc = psum1.tile([128, 512], F32, tag="pc", name="pcG")
        nc.tensor.matmul(pc[0:32, 0:2], _w1(wt, "fc1_blkf"), am[:],
                         start=True, stop=True)
        fcr = smalls.tile([32, 2], F32, tag="fcr", name="fcr")
        nc.scalar.activation(fcr[:], pc[0:32, 0:2], AF.Relu)
        pc = psum1.tile([128, 512], F32, tag="pc", name="pcH")
        nc.tensor.matmul(pc[:, 0:2], _w1(wt, "fc2_blkf"), fcr[:],
                         start=True, stop=True)
        cs = smalls.tile([128, 1], F32, tag="cs", name="cs")
        nc.vector.tensor_reduce(cs[:], pc[:, 0:2], AX.X, ALU.add)
        nc.scalar.activation(pt["ca"][:], cs[:], AF.Sigmoid)

    for ch in range(NCH):
        units.append(inv_chunk(ch))
    units.append(coord)
    units.append(chattn)
    return units


def _xwin(pt, row0, col, nrows):
    """Window [128, nrows, 64] at slab row row0, col (absolute incl. margin),
    choosing the even/odd slab for 4B-aligned bf16 starts."""
    if col % 2 == 0:
        return pt["x2ne"][:, row0:row0 + nrows, col:col + W]
    return pt["x2no"][:, row0:row0 + nrows, col - 1:col - 1 + W]


def _ph2(tc, nc, p, pt, xr2d, wt, plans, uscr, rscr, ph2p, psamp, st,
         interleave=(), utp=None):
    entries = plans[p]
    dcn = _w2(wt, "dcn_blk")
    pending = list(interleave)
    slots = [18]

    def pop_units():
        if pending and slots[0] > 0:
            n = max(1, (len(pending) + slots[0] - 1) // slots[0])
            for _ in range(n):
                if pending:
                    pending.pop(0)()
        slots[0] -= 1

    for half in range(2):
        banks = [psamp.tile([128, 1024], F32, tag=f"bk{c}", name=f"bk{c}")
                 for c in range(2)]
        # clip rare entries to this half
        clips = []
        for e, (sl, k, sy, sx, ya, yb, ak, bk) in enumerate(entries):
            a = max(ya, 32 * half)
            b = min(yb, 32 * half + 32)
            if a < b:
                clips.append((e, sl, k, sy, sx, a, b))
        nterms = 81 * 4 + sum(1 for _ in clips)  # per-bank counting via ti
        ti = 0
        last_core = (8, 8)
        for k in range(9):
            ky, kx = k // 3 - 1, k % 3 - 1
            if k == 8:
                for (e, sl, k_e, sy, sx, a, b) in clips:
                    ke_y, ke_x = k_e // 3 - 1, k_e % 3 - 1
                    ny = b - a
                    ubc = ph2p.tile([128, 32, W], BF16, tag="ubc",
                                    name="ubc")
                    _bcast_dma(nc, st,
                               ubc[64 * sl:64 * sl + 64, 0:ny, :],
                               rscr[p][e:e + 1, a * W:b * W]
                               .rearrange("o (h w) -> o h w", w=W)
                               .partition_broadcast(64))
                    cpr = ph2p.tile([128, 32, W], BF16, tag="cpr",
                                    name="cpr")
                    xw = _xwin(pt, YM + a + ke_y + sy, XM + ke_x + sx, ny)
                    nc.vector.tensor_tensor(
                        cpr[64 * sl:64 * sl + 64, 0:ny, :],
                        ubc[64 * sl:64 * sl + 64, 0:ny, :],
                        xw[64 * sl:64 * sl + 64, :, :], ALU.mult)
                    r0 = a - 32 * half
                    r1 = b - 32 * half
                    q0, q1 = r0 // 8, (r1 - 1) // 8
                    for q in range(q0, q1 + 1):
                        ra = max(r0, q * 8)
                        rb = min(r1, q * 8 + 8)
                        c = q // 2
                        nc.tensor.matmul(
                            banks[c][:, (ra - c * 16) * W:(rb - c * 16) * W],
                            dcn[:, k_e, :][64 * sl:64 * sl + 64, :],
                            cpr[64 * sl:64 * sl + 64,
                                ra - r0:rb - r0, :],
                            start=False, stop=False)
            for s in range(9):
                sy, sx = s // 3 - 1, s % 3 - 1
                r = s * 9 + k
                ut = utp.tile([128, 32, W], BF16, tag="ut", name="ut")
                if MERGED_BCAST:
                    _bcast_dma(nc, st, ut[:],
                               uscr[p][:, r:r + 1,
                                       half * 2048:half * 2048 + 2048]
                               .rearrange("s o (h w) -> s o h w", w=W)
                               .broadcast_to([2, 64, 32, W]))
                else:
                    for sl in range(2):
                        _bcast_dma(nc, st,
                                   ut[64 * sl:64 * sl + 64, :, :],
                                   uscr[p][sl, r:r + 1,
                                           half * 2048:half * 2048 + 2048]
                                   .rearrange("o (h w) -> o h w", w=W)
                                   .partition_broadcast(64))
                prod = ph2p.tile([128, 32, W], BF16, tag="prod", name="prod")
                xw = _xwin(pt, YM + 32 * half + ky + sy, XM + kx + sx, 32)
                nc.vector.tensor_tensor(prod[:], ut[:], xw, ALU.mult)
                first = (k == 0 and s == 0)
                last = (k == 8 and s == 8)
                for c in range(2):
                    for q in range(2):
                        nc.tensor.matmul(
                            banks[c][:, q * 512:(q + 1) * 512], dcn[:, k, :],
                            prod[:, c * 16 + q * 8:c * 16 + q * 8 + 8, :],
                            start=first, stop=last)
            pop_units()
        for c in range(2):
            nc.scalar.activation(
                xr2d[:, half * 2048 + c * 1024:half * 2048 + (c + 1) * 1024],
                banks[c][:], AF.Relu, bias=_w1(wt, "dcn_b_pk"))
    while pending:
        pending.pop(0)()


def _ph3(tc, nc, p, pt, xr2d, xin, yout, ph3p):
    s0 = 2 * p
    for hf in range(2):
        cols = slice(hf * 2048, (hf + 1) * 2048)
        gxr = ph3p.tile([128, 2048], F32, tag="gxr", name=f"gxr{p}{hf}")
        for sl in range(2):
            nc.sync.dma_start(gxr[64 * sl:64 * sl + 64, :],
                              xin[s0 + sl][:, cols])
        out2 = ph3p.tile([128, 2048], BF16, tag="o2", name=f"o2{p}{hf}")
        nc.vector.tensor_tensor(
            out2[:], xr2d[:, cols],
            pt["ca"][:].broadcast_to([128, 2048]), ALU.mult)
        nc.vector.tensor_tensor(out2[:], out2[:], pt["out0"][:, cols],
                                ALU.add)
        nc.scalar.activation(out2[:], out2[:], AF.Sigmoid)
        nc.vector.tensor_tensor(gxr[:], gxr[:], out2[:], ALU.mult)
        for sl in range(2):
            nc.sync.dma_start(yout[s0 + sl][:, cols],
                              gxr[64 * sl:64 * sl + 64, :])


# ---------------------------------------------------------------------------
# entry point
# ---------------------------------------------------------------------------

_CACHE = {}


def _prep_all(inputs):
    x = np.asarray(inputs["x"], np.float32)
    assert x.shape == (2, 1024, 64, 64)
    x_slices = np.ascontiguousarray(x.reshape(32, 64, HW))
    wd = _host_prep(inputs)
    off = _host_offsets(x_slices, wd)
    plans_all = _plan_rare(off)          # 16 pairs (32 slices)
    return x_slices, wd, plans_all


USE_RARE = False          # rare ring corrections (cross-core union program)


def kernel(**inputs):
    x_slices, wd, plans_all = _prep_all(inputs)
    x16 = x_slices.astype(ml_dtypes.bfloat16)

    # One SPMD program for all cores: plan structure must be core-uniform,
    # so use the empty plan (relu-clipped hats only; rel err ~6e-3).
    plans = [[], []]

    key = (repr(plans),)
    if key not in _CACHE:
        _CACHE[key] = (build_nc(wd, plans), _build_blobs(wd, plans))
    nc, (b1, b2, _, _, _, _) = _CACHE[key]
    in_maps = []
    for core in range(NCORES):
        in_maps.append({
            "xin": np.ascontiguousarray(
                x_slices[core * NSLICES:(core + 1) * NSLICES]),
            "xin16": np.ascontiguousarray(
                x16[core * NSLICES:(core + 1) * NSLICES]),
            "wblob": b1,
            "wblob16": b2,
        })
    res = run_bass_kernel_spmd(nc, in_maps, list(range(NCORES))).results
    out = np.empty((32, 64, HW), np.float32)
    for core in range(NCORES):
        out[core * NSLICES:(core + 1) * NSLICES] = res[core]["yout"]
    return out.reshape(2, 1024, 64, 64)


if __name__ == "__main__":
    import reference
    inputs = {k: np.asarray(v) for k, v in reference.setup_inputs().items()}
    got = kernel(**inputs)
    print("kernel output:", got.shape, got.dtype)
